# revision 32
# baseline (speedup 1.0000x reference)
# Trainium2 Bass kernel for nn_LogitsNew (dense_mlp).
#
#   u = gelu(x @ W_proj + b_proj)                       [B, D]
#   logits = (u @ W_u)[:, None, :] + ee @ W_e           [B, N, C]
#
# Sharding: data-parallel over batch B across 8 cores (4 batches/core).
#
# fp16 end-to-end (tolerance 2e-2; measured rel err ~4e-4), host-side
# layout transforms (k-chunk layouts, no PE transposes for the main path),
# fp16 stores (upcast on host).  ~10MB HBM traffic per core; the two
# HWDGE rings sustain ~175GB/s each under 8-core contention, so all
# transfers are large (0.5-1MB) and packed by consumption deadline.
# Weights are sliced by OUTPUT half (ch) so each consumer needs only the
# half that has landed.
#
# PE order:  warmup | A(mt0+mt1, ch0) | z-ch0 | A(ch1) | mt2 | z-ch1 |
#            mt3 | uT | y-ch1 | y-ch0 | ybc | [DVE epilogue mt0-3] |
#            mt4..7 (sel-fused).
#   - warmup/fill matmuls on the identity keep the tensor-engine DVFS
#     clock ramped while the first MBs stream in (a cold/idle PE runs
#     matmuls 2-3x slower for ~3.5us).
#   - y broadcast is done on the PE: late m-tiles append a selector
#     matmul (lhsT = e_b x ones_128, rhs = y[4, 512] fp16) to their PSUM
#     group, so PSUM holds final logits; early m-tiles get ybc (4 PE
#     broadcast-matmuls) added on the DVE in a hoisted epilogue.

import sys

if "/opt/trn_rl_repo" not in sys.path:
    sys.path.insert(0, "/opt/trn_rl_repo")

import numpy as np

import concourse.bass as bass
import concourse.mybir as mybir
import concourse.tile as tile
from concourse import bacc
from concourse.bass_utils import run_bass_kernel_spmd
from concourse.masks import make_identity

P = 128
B, N, D, C = 32, 256, 1024, 1024
NCORES = 8
BPC = B // NCORES          # batches per core
KT = D // P                # 8 k-tiles over the contraction dim
FD = 512                   # matmul moving free dim (one PSUM bank of fp32)
NT = N // P                # 2 n-tiles per batch
MT = BPC * NT              # 8 m-tiles per core
NEARLY = 4                 # m-tiles drained before y exists (epilogue add)
NBCAST = (NEARLY + NT - 1) // NT   # batches needing a broadcast y

F32 = mybir.dt.float32
F16 = mybir.dt.float16
GELU = mybir.ActivationFunctionType.Gelu

_CACHE = {}


def _build():
    if "nc" in _CACHE:
        return _CACHE["nc"]

    nc = bacc.Bacc("TRN2", target_bir_lowering=False, debug=False, num_devices=NCORES)

    # host-transformed inputs (fp16; eet partition-major, weights ch-sliced)
    eet = nc.dram_tensor("eet", [P, BPC, KT, N], F16, kind="ExternalInput").ap()
    we = nc.dram_tensor("we", [P, 2, KT, FD], F16, kind="ExternalInput").ap()
    wu = nc.dram_tensor("wu", [P, 2, KT, FD], F16, kind="ExternalInput").ap()
    wp = nc.dram_tensor("wp", [P, 2, KT, FD], F16, kind="ExternalInput").ap()
    xt = nc.dram_tensor("xt", [P, KT, BPC], F16, kind="ExternalInput").ap()
    bp = nc.dram_tensor("bp", [1, D], F16, kind="ExternalInput").ap()
    seld = nc.dram_tensor("sel", [BPC, BPC * P], F16, kind="ExternalInput").ap()
    out = nc.dram_tensor("logits", [BPC, N, C], F16, kind="ExternalOutput").ap()

    with tile.TileContext(nc) as tc:
        with (
            tc.tile_pool(name="const", bufs=1) as cpool,
            tc.tile_pool(name="outs", bufs=1) as outpool,
            tc.tile_pool(name="ost", bufs=4) as ostpool,
            tc.tile_pool(name="tp_ps", bufs=1, space="PSUM") as tp_ps,
            tc.tile_pool(name="warm_ps", bufs=1, space="PSUM") as warm_ps,
            tc.tile_pool(name="mm_ps", bufs=6, space="PSUM") as mm_ps,
        ):
            # ---- sync ring: we_ch0 | wp_ch0 | wu_ch0 | ee23 | stores ----
            wech, wpch, wuch = [None, None], [None, None], [None, None]
            wech[0] = cpool.tile([P, KT, FD], F16, name="we_0")
            nc.sync.dma_start(wech[0], we[:, 0])
            wpch[0] = cpool.tile([P, KT, FD], F16, name="wp_0")
            nc.sync.dma_start(wpch[0], wp[:, 0])
            wuch[0] = cpool.tile([P, KT, FD], F16, name="wu_0")
            nc.sync.dma_start(wuch[0], wu[:, 0])

            # ---- scalar ring: ee0 | x/b/sel | we_ch1 | ee1 | wp_ch1 |
            # wu_ch1 | stores ----
            ee0 = cpool.tile([P, KT, N], F16, name="ee_0")
            nc.scalar.dma_start(ee0, eet[:, 0])
            xsb = cpool.tile([P, KT, BPC], F16)
            nc.scalar.dma_start(xsb, xt)
            bsb = cpool.tile([1, D], F16)
            nc.scalar.dma_start(bsb, bp)
            sel = cpool.tile([BPC, BPC * P], F16)
            nc.scalar.dma_start(sel, seld)
            wech[1] = cpool.tile([P, KT, FD], F16, name="we_1")
            nc.scalar.dma_start(wech[1], we[:, 1])
            ee1 = cpool.tile([P, KT, N], F16, name="ee_1")
            nc.scalar.dma_start(ee1, eet[:, 1])
            ee23 = cpool.tile([P, 2, KT, N], F16)
            nc.sync.dma_start(ee23, eet[:, 2:4])
            wpch[1] = cpool.tile([P, KT, FD], F16, name="wp_1")
            nc.scalar.dma_start(wpch[1], wp[:, 1])
            wuch[1] = cpool.tile([P, KT, FD], F16, name="wu_1")
            nc.scalar.dma_start(wuch[1], wu[:, 1])

            def ee_at(b, k):
                if b == 0:
                    return ee0[:, k]
                if b == 1:
                    return ee1[:, k]
                return ee23[:, b - 2, k]

            # ---- constants (copies on the vector engine: the scalar queue
            # starts with ~4us of DMA issues) ----
            ident_f = cpool.tile([P, P], F32)
            make_identity(nc, ident_f)
            ident = cpool.tile([P, P], F16)
            nc.vector.tensor_copy(ident, ident_f)
            ones_f = cpool.tile([1, P], F32)
            nc.gpsimd.memset(ones_f, 1.0)
            ones = cpool.tile([1, P], F16)
            nc.vector.tensor_copy(ones, ones_f)

            warm = warm_ps.tile([P, P], F32, tag="warm")

            def warmf(n):
                for _ in range(n):
                    nc.tensor.matmul(warm, ident, ident, start=True, stop=True)

            warmf(70)

            zsb = cpool.tile([BPC, C], F16)
            uT = cpool.tile([P, KT * BPC], F16)
            ysb = cpool.tile([BPC, C], F16)
            ybc = cpool.tile([P, NBCAST, C], F32)
            o32 = []

            # ---- A-block: mt0 + mt1, one ch at a time, k-major; z spliced
            # into the DMA-fill window ----
            mpsA = [
                [
                    mm_ps.tile([P, FD], F32, tag="mm", name=f"mm_{mt}_{ch}")
                    for ch in range(2)
                ]
                for mt in range(2)
            ]
            zps = [None, None]

            def a_half(ch):
                cs = slice(ch * FD, (ch + 1) * FD)
                for k in range(KT):
                    for mt in range(2):
                        nc.tensor.matmul(
                            mpsA[mt][ch],
                            ee0[:, k][:, mt * P : (mt + 1) * P],
                            wech[ch][:, k],
                            start=(k == 0),
                            stop=(k == KT - 1),
                        )

            def z_half(ch):
                cs = slice(ch * FD, (ch + 1) * FD)
                zp = mm_ps.tile([P, FD], F32, tag="mm", name=f"z_{ch}")
                for k in range(KT):
                    nc.tensor.matmul(
                        zp[:BPC], xsb[:, k, :], wpch[ch][:, k],
                        start=(k == 0), stop=False,
                    )
                nc.tensor.matmul(
                    zp[:BPC], ones[:1, :BPC], bsb[:1, cs],
                    start=False, stop=True,
                )
                nc.vector.tensor_copy(zsb[:, cs], zp[:BPC])

            def transposes():
                tp = tp_ps.tile([P, KT * BPC], F16, tag="tp")
                for k in range(KT):
                    nc.tensor.transpose(
                        tp[:, k * BPC : (k + 1) * BPC],
                        zsb[:BPC, k * P : (k + 1) * P],
                        ident[:BPC, :BPC],
                    )
                nc.scalar.activation(uT, tp, GELU)

            def y_half(ch):
                cs = slice(ch * FD, (ch + 1) * FD)
                yp = mm_ps.tile([P, FD], F32, tag="mm", name=f"y_{ch}")
                for k in range(KT):
                    nc.tensor.matmul(
                        yp[:BPC], uT[:, k * BPC : (k + 1) * BPC],
                        wuch[ch][:, k],
                        start=(k == 0), stop=(k == KT - 1),
                    )
                nc.vector.tensor_copy(ysb[:, cs], yp[:BPC])

            def ybc_mms():
                for b2 in range(NBCAST):
                    for ch in range(2):
                        cs = slice(ch * FD, (ch + 1) * FD)
                        bp_ = mm_ps.tile([P, FD], F32, tag="mm", name=f"yb{b2}{ch}")
                        nc.tensor.matmul(
                            bp_, sel[:, b2 * P : (b2 + 1) * P], ysb[:BPC, cs],
                            start=True, stop=True,
                        )
                        if ch == 0:
                            nc.scalar.copy(ybc[:, b2, cs], bp_)
                        else:
                            nc.vector.tensor_copy(ybc[:, b2, cs], bp_)

            def store(mt, o):
                b, nh = divmod(mt, NT)
                ns = slice(nh * P, (nh + 1) * P)
                if mt >= MT - 2:
                    # tail tiles: split the store across both rings so the
                    # last transfer is half-size
                    ov = out[b, ns, :].rearrange("p (a f) -> p a f", a=2)
                    nc.sync.dma_start(ov[:, 0], o[:, 0, :])
                    nc.scalar.dma_start(ov[:, 1], o[:, 1, :])
                else:
                    eng = nc.sync if mt % 2 == 0 else nc.scalar
                    eng.dma_start(out[b, ns, :], o.rearrange("p a f -> p (a f)"))

            def main_mt(mt):
                b, nh = divmod(mt, NT)
                ns = slice(nh * P, (nh + 1) * P)
                fuse_y = mt >= NEARLY
                mps = [
                    mm_ps.tile([P, FD], F32, tag="mm", name=f"mm_{mt}_{ch}")
                    for ch in range(2)
                ]
                for ch in range(2):
                    cs = slice(ch * FD, (ch + 1) * FD)
                    for k in range(KT):
                        nc.tensor.matmul(
                            mps[ch],
                            ee_at(b, k)[:, ns],
                            wech[ch][:, k],
                            start=(k == 0),
                            stop=(False if fuse_y else k == KT - 1),
                        )
                    if fuse_y:
                        nc.tensor.matmul(
                            mps[ch], sel[:, b * P : (b + 1) * P], ysb[:BPC, cs],
                            start=False, stop=True,
                        )
                if mt < NEARLY:
                    o = outpool.tile([P, 2, FD], F32, tag=f"o{mt}")
                    nc.scalar.copy(o[:, 0, :], mps[0])
                    nc.vector.tensor_copy(o[:, 1, :], mps[1])
                    o32.append(o)
                else:
                    o = ostpool.tile([P, 2, FD], F16, tag="ost", name=f"ost{mt}")
                    nc.scalar.copy(o[:, 0, :], mps[0])
                    nc.vector.tensor_copy(o[:, 1, :], mps[1])
                    store(mt, o)

            # ---- the schedule ----
            a_half(0)
            warmf(8)
            a_half(1)
            for mt in range(2):
                o = outpool.tile([P, 2, FD], F32, tag=f"o{mt}")
                nc.scalar.copy(o[:, 0, :], mpsA[mt][0])
                nc.vector.tensor_copy(o[:, 1, :], mpsA[mt][1])
                o32.append(o)
            z_half(0)
            main_mt(2)
            z_half(1)
            main_mt(3)
            transposes()
            warmf(6)
            y_half(0)
            y_half(1)
            ybc_mms()
            # hoisted epilogue: add y to the early tiles on the DVE and
            # store them, while the PE works on mt4..7
            for emt in range(NEARLY):
                eb = emt // NT
                o = ostpool.tile([P, 2, FD], F16, tag="ost", name=f"oste{emt}")
                nc.vector.tensor_add(o[:, 0, :], o32[emt][:, 0, :], ybc[:, eb, 0:FD])
                nc.vector.tensor_add(o[:, 1, :], o32[emt][:, 1, :], ybc[:, eb, FD:C])
                store(emt, o)
            for mt in range(4, MT):
                main_mt(mt)

    nc.compile()
    _CACHE["nc"] = nc
    return nc


def _prep(inputs):
    """Host-side cast to fp16 + layout transforms."""
    x = np.asarray(inputs["encoded_utterance"], np.float32)
    ee = np.asarray(inputs["element_embeddings"], np.float32)
    w = np.asarray(inputs["weight_matrix"], np.float32)
    wp = np.asarray(inputs["W_proj"], np.float32)
    bp = np.asarray(inputs["b_proj"], np.float32).reshape(1, D)

    # eet[p, b, k, n] = ee[b, n, k*128+p]  (partition-major)
    eet = np.ascontiguousarray(
        ee.reshape(B, N, KT, P).transpose(3, 0, 2, 1)
    ).astype(np.float16)

    # ch-sliced k-chunks: W2[p, ch, k, c'] = W[k*128+p, ch*512+c']
    def chkchunk(m):
        return np.ascontiguousarray(
            m.reshape(KT, P, 2, FD).transpose(1, 2, 0, 3)
        ).astype(np.float16)

    we_h = chkchunk(w[D:])
    wu_h = chkchunk(w[:D])
    wp_h = chkchunk(wp)
    bp_h = bp.astype(np.float16)
    # xt[p, k, b] = x[b, k*128+p], per-core slice of b
    xt_full = np.ascontiguousarray(
        x.reshape(B, KT, P).transpose(2, 1, 0)
    ).astype(np.float16)
    sel_h = np.kron(np.eye(BPC), np.ones((1, P))).astype(np.float16)
    return eet, we_h, wu_h, wp_h, bp_h, xt_full, sel_h


def run(inputs, trace=False, **kwargs):
    nc = _build()
    eet, we_h, wu_h, wp_h, bp_h, xt_full, sel_h = _prep(inputs)

    in_maps = []
    for i in range(NCORES):
        bs = slice(i * BPC, (i + 1) * BPC)
        in_maps.append(
            {
                "eet": np.ascontiguousarray(eet[:, bs]),
                "we": we_h,
                "wu": wu_h,
                "wp": wp_h,
                "xt": np.ascontiguousarray(xt_full[:, :, bs]),
                "bp": bp_h,
                "sel": sel_h,
            }
        )

    res = run_bass_kernel_spmd(
        nc, in_maps, core_ids=list(range(NCORES)), trace=trace, **kwargs
    )
    full = np.concatenate([r["logits"] for r in res.results], axis=0)
    return full.astype(np.float32), res


def kernel(**inputs) -> np.ndarray:
    return run(inputs, trace=False)[0]


# revision 33
# speedup vs baseline: 1.0074x; 1.0074x over previous
# Trainium2 Bass kernel for nn_LogitsNew (dense_mlp).
#
#   u = gelu(x @ W_proj + b_proj)                       [B, D]
#   logits = (u @ W_u)[:, None, :] + ee @ W_e           [B, N, C]
#
# Sharding: data-parallel over batch B across 8 cores (4 batches/core).
#
# fp16 end-to-end (tolerance 2e-2; measured rel err ~4e-4), host-side
# layout transforms (k-chunk layouts, no PE transposes for the main path),
# fp16 stores (upcast on host).  ~10MB HBM traffic per core; the two
# HWDGE rings sustain ~175GB/s each under 8-core contention, so all
# transfers are large (0.5-1MB) and packed by consumption deadline.
# Weights are sliced by OUTPUT half (ch) so each consumer needs only the
# half that has landed.
#
# PE order:  warmup | A(mt0+mt1, ch0) | z-ch0 | A(ch1) | mt2 | z-ch1 |
#            mt3 | uT | y-ch1 | y-ch0 | ybc | [DVE epilogue mt0-3] |
#            mt4..7 (sel-fused).
#   - warmup/fill matmuls on the identity keep the tensor-engine DVFS
#     clock ramped while the first MBs stream in (a cold/idle PE runs
#     matmuls 2-3x slower for ~3.5us).
#   - y broadcast is done on the PE: late m-tiles append a selector
#     matmul (lhsT = e_b x ones_128, rhs = y[4, 512] fp16) to their PSUM
#     group, so PSUM holds final logits; early m-tiles get ybc (4 PE
#     broadcast-matmuls) added on the DVE in a hoisted epilogue.

import sys

if "/opt/trn_rl_repo" not in sys.path:
    sys.path.insert(0, "/opt/trn_rl_repo")

import numpy as np

import concourse.bass as bass
import concourse.mybir as mybir
import concourse.tile as tile
from concourse import bacc
from concourse.bass_utils import run_bass_kernel_spmd
from concourse.masks import make_identity

P = 128
B, N, D, C = 32, 256, 1024, 1024
NCORES = 8
BPC = B // NCORES          # batches per core
KT = D // P                # 8 k-tiles over the contraction dim
FD = 512                   # matmul moving free dim (one PSUM bank of fp32)
NT = N // P                # 2 n-tiles per batch
MT = BPC * NT              # 8 m-tiles per core
NEARLY = 4                 # m-tiles drained before y exists (epilogue add)
NBCAST = (NEARLY + NT - 1) // NT   # batches needing a broadcast y

F32 = mybir.dt.float32
F16 = mybir.dt.float16
GELU = mybir.ActivationFunctionType.Gelu

_CACHE = {}


def _build():
    if "nc" in _CACHE:
        return _CACHE["nc"]

    nc = bacc.Bacc("TRN2", target_bir_lowering=False, debug=False, num_devices=NCORES)

    # host-transformed inputs (fp16; eet partition-major, weights ch-sliced)
    eet = nc.dram_tensor("eet", [P, BPC, KT, N], F16, kind="ExternalInput").ap()
    we = nc.dram_tensor("we", [P, 2, KT, FD], F16, kind="ExternalInput").ap()
    wu = nc.dram_tensor("wu", [P, 2, KT, FD], F16, kind="ExternalInput").ap()
    wp = nc.dram_tensor("wp", [P, 2, KT, FD], F16, kind="ExternalInput").ap()
    xt = nc.dram_tensor("xt", [P, KT, BPC], F16, kind="ExternalInput").ap()
    bp = nc.dram_tensor("bp", [1, D], F16, kind="ExternalInput").ap()
    seld = nc.dram_tensor("sel", [BPC, BPC * P], F16, kind="ExternalInput").ap()
    out = nc.dram_tensor("logits", [BPC, N, C], F16, kind="ExternalOutput").ap()

    with tile.TileContext(nc) as tc:
        with (
            tc.tile_pool(name="const", bufs=1) as cpool,
            tc.tile_pool(name="outs", bufs=1) as outpool,
            tc.tile_pool(name="ost", bufs=4) as ostpool,
            tc.tile_pool(name="tp_ps", bufs=1, space="PSUM") as tp_ps,
            tc.tile_pool(name="warm_ps", bufs=1, space="PSUM") as warm_ps,
            tc.tile_pool(name="mm_ps", bufs=6, space="PSUM") as mm_ps,
        ):
            # ---- sync ring: we_ch0 | wp_ch0 | wu_ch0 | ee23 | stores ----
            wech, wpch, wuch = [None, None], [None, None], [None, None]
            wech[0] = cpool.tile([P, KT, FD], F16, name="we_0")
            nc.sync.dma_start(wech[0], we[:, 0])
            wpch[0] = cpool.tile([P, KT, FD], F16, name="wp_0")
            nc.sync.dma_start(wpch[0], wp[:, 0])
            wuch[0] = cpool.tile([P, KT, FD], F16, name="wu_0")
            nc.sync.dma_start(wuch[0], wu[:, 0])

            # ---- scalar ring: ee0 | x/b/sel | we_ch1 | ee1 | wp_ch1 |
            # wu_ch1 | stores ----
            ee0 = cpool.tile([P, KT, N], F16, name="ee_0")
            nc.scalar.dma_start(ee0, eet[:, 0])
            xsb = cpool.tile([P, KT, BPC], F16)
            nc.scalar.dma_start(xsb, xt)
            bsb = cpool.tile([1, D], F16)
            nc.scalar.dma_start(bsb, bp)
            sel = cpool.tile([BPC, BPC * P], F16)
            nc.scalar.dma_start(sel, seld)
            wech[1] = cpool.tile([P, KT, FD], F16, name="we_1")
            nc.scalar.dma_start(wech[1], we[:, 1])
            ee1 = cpool.tile([P, KT, N], F16, name="ee_1")
            nc.scalar.dma_start(ee1, eet[:, 1])
            ee23 = cpool.tile([P, 2, KT, N], F16)
            nc.sync.dma_start(ee23, eet[:, 2:4])
            wpch[1] = cpool.tile([P, KT, FD], F16, name="wp_1")
            nc.scalar.dma_start(wpch[1], wp[:, 1])
            wuch[1] = cpool.tile([P, KT, FD], F16, name="wu_1")
            nc.scalar.dma_start(wuch[1], wu[:, 1])

            def ee_at(b, k):
                if b == 0:
                    return ee0[:, k]
                if b == 1:
                    return ee1[:, k]
                return ee23[:, b - 2, k]

            # ---- constants (copies on the vector engine: the scalar queue
            # starts with ~4us of DMA issues) ----
            ident_f = cpool.tile([P, P], F32)
            make_identity(nc, ident_f)
            ident = cpool.tile([P, P], F16)
            nc.vector.tensor_copy(ident, ident_f)
            ones_f = cpool.tile([1, P], F32)
            nc.gpsimd.memset(ones_f, 1.0)
            ones = cpool.tile([1, P], F16)
            nc.vector.tensor_copy(ones, ones_f)

            warm = warm_ps.tile([P, P], F32, tag="warm")

            def warmf(n):
                for _ in range(n):
                    nc.tensor.matmul(warm, ident, ident, start=True, stop=True)

            warmf(70)

            zsb = cpool.tile([BPC, C], F16)
            uT = cpool.tile([P, KT * BPC], F16)
            ysb = cpool.tile([BPC, C], F16)
            ybc = cpool.tile([P, NBCAST, C], F32)
            o32 = []

            # ---- A-block: mt0 + mt1, one ch at a time, k-major; z spliced
            # into the DMA-fill window ----
            mpsA = [
                [
                    mm_ps.tile([P, FD], F32, tag="mm", name=f"mm_{mt}_{ch}")
                    for ch in range(2)
                ]
                for mt in range(2)
            ]
            zps = [None, None]

            def a_half(ch):
                cs = slice(ch * FD, (ch + 1) * FD)
                for k in range(KT):
                    for mt in range(2):
                        nc.tensor.matmul(
                            mpsA[mt][ch],
                            ee0[:, k][:, mt * P : (mt + 1) * P],
                            wech[ch][:, k],
                            start=(k == 0),
                            stop=(k == KT - 1),
                        )

            def z_half(ch):
                cs = slice(ch * FD, (ch + 1) * FD)
                zp = mm_ps.tile([P, FD], F32, tag="mm", name=f"z_{ch}")
                for k in range(KT):
                    nc.tensor.matmul(
                        zp[:BPC], xsb[:, k, :], wpch[ch][:, k],
                        start=(k == 0), stop=False,
                    )
                nc.tensor.matmul(
                    zp[:BPC], ones[:1, :BPC], bsb[:1, cs],
                    start=False, stop=True,
                )
                nc.vector.tensor_copy(zsb[:, cs], zp[:BPC])

            def transposes():
                tp = tp_ps.tile([P, KT * BPC], F16, tag="tp")
                for k in range(KT):
                    nc.tensor.transpose(
                        tp[:, k * BPC : (k + 1) * BPC],
                        zsb[:BPC, k * P : (k + 1) * P],
                        ident[:BPC, :BPC],
                    )
                nc.scalar.activation(uT, tp, GELU)

            def y_half(ch):
                cs = slice(ch * FD, (ch + 1) * FD)
                yp = mm_ps.tile([P, FD], F32, tag="mm", name=f"y_{ch}")
                for k in range(KT):
                    nc.tensor.matmul(
                        yp[:BPC], uT[:, k * BPC : (k + 1) * BPC],
                        wuch[ch][:, k],
                        start=(k == 0), stop=(k == KT - 1),
                    )
                nc.vector.tensor_copy(ysb[:, cs], yp[:BPC])

            def ybc_mms():
                for b2 in range(NBCAST):
                    for ch in range(2):
                        cs = slice(ch * FD, (ch + 1) * FD)
                        bp_ = mm_ps.tile([P, FD], F32, tag="mm", name=f"yb{b2}{ch}")
                        nc.tensor.matmul(
                            bp_, sel[:, b2 * P : (b2 + 1) * P], ysb[:BPC, cs],
                            start=True, stop=True,
                        )
                        if ch == 0:
                            nc.scalar.copy(ybc[:, b2, cs], bp_)
                        else:
                            nc.vector.tensor_copy(ybc[:, b2, cs], bp_)

            def store(mt, o):
                b, nh = divmod(mt, NT)
                ns = slice(nh * P, (nh + 1) * P)
                if mt >= MT - 2:
                    # tail tiles: split the store across both rings so the
                    # last transfer is half-size
                    ov = out[b, ns, :].rearrange("p (a f) -> p a f", a=2)
                    nc.sync.dma_start(ov[:, 0], o[:, 0, :])
                    nc.scalar.dma_start(ov[:, 1], o[:, 1, :])
                else:
                    eng = nc.sync if mt % 2 == 0 else nc.scalar
                    eng.dma_start(out[b, ns, :], o.rearrange("p a f -> p (a f)"))

            def main_mt(mt):
                b, nh = divmod(mt, NT)
                ns = slice(nh * P, (nh + 1) * P)
                fuse_y = mt >= NEARLY
                mps = [
                    mm_ps.tile([P, FD], F32, tag="mm", name=f"mm_{mt}_{ch}")
                    for ch in range(2)
                ]
                for ch in range(2):
                    cs = slice(ch * FD, (ch + 1) * FD)
                    for k in range(KT):
                        nc.tensor.matmul(
                            mps[ch],
                            ee_at(b, k)[:, ns],
                            wech[ch][:, k],
                            start=(k == 0),
                            stop=(False if fuse_y else k == KT - 1),
                        )
                    if fuse_y:
                        nc.tensor.matmul(
                            mps[ch], sel[:, b * P : (b + 1) * P], ysb[:BPC, cs],
                            start=False, stop=True,
                        )
                if mt < NEARLY:
                    o = outpool.tile([P, 2, FD], F32, tag=f"o{mt}")
                    nc.scalar.copy(o[:, 0, :], mps[0])
                    nc.vector.tensor_copy(o[:, 1, :], mps[1])
                    o32.append(o)
                else:
                    o = ostpool.tile([P, 2, FD], F16, tag="ost", name=f"ost{mt}")
                    nc.scalar.copy(o[:, 0, :], mps[0])
                    nc.vector.tensor_copy(o[:, 1, :], mps[1])
                    store(mt, o)

            # ---- the schedule ----
            a_half(0)
            warmf(14)
            a_half(1)
            for mt in range(2):
                o = outpool.tile([P, 2, FD], F32, tag=f"o{mt}")
                nc.scalar.copy(o[:, 0, :], mpsA[mt][0])
                nc.vector.tensor_copy(o[:, 1, :], mpsA[mt][1])
                o32.append(o)
            z_half(0)
            main_mt(2)
            z_half(1)
            main_mt(3)
            transposes()
            warmf(6)
            y_half(0)
            y_half(1)
            ybc_mms()
            # hoisted epilogue: add y to the early tiles on the DVE and
            # store them, while the PE works on mt4..7
            for emt in range(NEARLY):
                eb = emt // NT
                o = ostpool.tile([P, 2, FD], F16, tag="ost", name=f"oste{emt}")
                nc.vector.tensor_add(o[:, 0, :], o32[emt][:, 0, :], ybc[:, eb, 0:FD])
                nc.vector.tensor_add(o[:, 1, :], o32[emt][:, 1, :], ybc[:, eb, FD:C])
                store(emt, o)
            for mt in range(4, MT):
                main_mt(mt)

    nc.compile()
    _CACHE["nc"] = nc
    return nc


def _prep(inputs):
    """Host-side cast to fp16 + layout transforms."""
    x = np.asarray(inputs["encoded_utterance"], np.float32)
    ee = np.asarray(inputs["element_embeddings"], np.float32)
    w = np.asarray(inputs["weight_matrix"], np.float32)
    wp = np.asarray(inputs["W_proj"], np.float32)
    bp = np.asarray(inputs["b_proj"], np.float32).reshape(1, D)

    # eet[p, b, k, n] = ee[b, n, k*128+p]  (partition-major)
    eet = np.ascontiguousarray(
        ee.reshape(B, N, KT, P).transpose(3, 0, 2, 1)
    ).astype(np.float16)

    # ch-sliced k-chunks: W2[p, ch, k, c'] = W[k*128+p, ch*512+c']
    def chkchunk(m):
        return np.ascontiguousarray(
            m.reshape(KT, P, 2, FD).transpose(1, 2, 0, 3)
        ).astype(np.float16)

    we_h = chkchunk(w[D:])
    wu_h = chkchunk(w[:D])
    wp_h = chkchunk(wp)
    bp_h = bp.astype(np.float16)
    # xt[p, k, b] = x[b, k*128+p], per-core slice of b
    xt_full = np.ascontiguousarray(
        x.reshape(B, KT, P).transpose(2, 1, 0)
    ).astype(np.float16)
    sel_h = np.kron(np.eye(BPC), np.ones((1, P))).astype(np.float16)
    return eet, we_h, wu_h, wp_h, bp_h, xt_full, sel_h


def run(inputs, trace=False, **kwargs):
    nc = _build()
    eet, we_h, wu_h, wp_h, bp_h, xt_full, sel_h = _prep(inputs)

    in_maps = []
    for i in range(NCORES):
        bs = slice(i * BPC, (i + 1) * BPC)
        in_maps.append(
            {
                "eet": np.ascontiguousarray(eet[:, bs]),
                "we": we_h,
                "wu": wu_h,
                "wp": wp_h,
                "xt": np.ascontiguousarray(xt_full[:, :, bs]),
                "bp": bp_h,
                "sel": sel_h,
            }
        )

    res = run_bass_kernel_spmd(
        nc, in_maps, core_ids=list(range(NCORES)), trace=trace, **kwargs
    )
    full = np.concatenate([r["logits"] for r in res.results], axis=0)
    return full.astype(np.float32), res


def kernel(**inputs) -> np.ndarray:
    return run(inputs, trace=False)[0]
